# revision 13
# baseline (speedup 1.0000x reference)
"""Trainium2 Bass kernel for nn_Attn: softmax(out_state @ (history @ W.T + b).T, axis=1).

Key algebra: E = out_state @ proj.T = (out_state @ W) @ history.T + (out_state @ b) 1^T.
The bias contributes a per-row constant, which softmax is invariant to, so it is
dropped entirely.  Per core (1/8 of out_state rows):
    A.T = W.T @ S.T        (PE, fp32r)
    E   = A @ H.T          (PE, fp32r; H transposed on-chip via PE transpose-mode)
    out = softmax(E, 1)    (DVE max/scale, ACT exp with per-row bias + sum accumulation)

fp32r operands are produced by SWDGE cast-DMAs (W, S, H, identity arrive
pre-rounded) or by the PSUM->SBUF copies that must happen anyway.
"""

import numpy as np

import concourse.bacc as bacc
import concourse.bass as bass
import concourse.tile as tile
from concourse import mybir
from concourse.bass_utils import run_bass_kernel_spmd

STATE, SEQ, HID = 4096, 8192, 1024
NCORES = 8
RPC = STATE // NCORES          # 512 out_state rows per core
ITILES = RPC // 128            # 4
KT = HID // 128                # 8 contraction tiles
SCHUNK = 512                   # seq columns per streamed chunk
NCHUNK = SEQ // SCHUNK         # 16
SSUB = SCHUNK // 128           # 4 row sub-tiles per chunk
NHALF = SEQ // 256             # H streamed in 256-row half-chunks

f32 = mybir.dt.float32
f32r = mybir.dt.float32r
AXX = mybir.AxisListType.X
EXP = mybir.ActivationFunctionType.Exp


def _build():
    nc = bacc.Bacc("TRN2", target_bir_lowering=False, debug=False)
    s_d = nc.dram_tensor("s", [RPC, HID], f32, kind="ExternalInput").ap()
    h_d = nc.dram_tensor("h", [SEQ, HID], f32, kind="ExternalInput").ap()
    w_d = nc.dram_tensor("w", [HID, HID], f32, kind="ExternalInput").ap()
    eye_d = nc.dram_tensor("eye", [128, 128], f32, kind="ExternalInput").ap()
    o_d = nc.dram_tensor("o", [RPC, SEQ], f32, kind="ExternalOutput").ap()

    with tile.TileContext(nc) as tc:
        with tc.tile_pool(name="persist", bufs=1) as persist, \
             tc.tile_pool(name="hraw", bufs=2) as hraw_p, \
             tc.tile_pool(name="htp", bufs=2) as ht_p, \
             tc.tile_pool(name="small", bufs=1) as small, \
             tc.tile_pool(name="tp_ps", bufs=4, space="PSUM") as tp_ps:

            # fp32r identity for transpose-mode matmuls
            ident = persist.tile([128, 128], f32r, name="ident")
            nc.gpsimd.dma_start(out=ident, in_=eye_d)
            # A.T laid out [k_partition, kt, i], fp32r (stationary operand of E matmuls)
            at_r = persist.tile([128, KT, RPC], f32r, name="at_r")

            half_tiles = {}

            def fetch_half(hh):
                hr = hraw_p.tile([128, 2, HID], f32r, name="hr")
                nc.gpsimd.dma_start(
                    out=hr,
                    in_=h_d[hh * 256:(hh + 1) * 256, :].rearrange(
                        "(a p) k -> p a k", p=128
                    ),
                )
                half_tiles[hh] = hr

            ht_tiles = {}

            def transpose_chunk(c):
                hr0 = half_tiles.pop(2 * c)
                hr1 = half_tiles.pop(2 * c + 1)
                ht = ht_p.tile([128, KT, SCHUNK], f32r, name="ht")
                for kb in range(KT):
                    tp = tp_ps.tile([128, SCHUNK], f32r, name="tp")
                    for a in range(SSUB):
                        src = hr0 if a < 2 else hr1
                        nc.tensor.transpose(
                            tp[:, a * 128:(a + 1) * 128],
                            src[:, a % 2, kb * 128:(kb + 1) * 128],
                            ident,
                        )
                    # split PSUM->SBUF copies between DVE (5) and ACT (3)
                    if kb in (1, 4, 6):
                        nc.scalar.copy(out=ht[:, kb, :], in_=tp)
                    else:
                        nc.vector.tensor_copy(ht[:, kb, :], tp)
                ht_tiles[c] = ht

            # ---------------- Phase A: A.T = W.T @ S.T ----------------
            with tc.tile_pool(name="pa", bufs=1) as pa, \
                 tc.tile_pool(name="pa_ps", bufs=2, space="PSUM") as paps:
                s_r = pa.tile([128, ITILES, HID], f32r, name="s_r")
                nc.gpsimd.dma_start(out=s_r, in_=s_d.rearrange("(a p) k -> p a k", p=128))
                w_r = pa.tile([128, KT, HID], f32r, name="w_r")
                # split so A.T for the low kt half can start sooner
                nc.gpsimd.dma_start(
                    out=w_r[:, :, 0:HID // 2],
                    in_=w_d[:, 0:HID // 2].rearrange("(t p) k -> p t k", p=128),
                )
                nc.gpsimd.dma_start(
                    out=w_r[:, :, HID // 2:],
                    in_=w_d[:, HID // 2:].rearrange("(t p) k -> p t k", p=128),
                )
                fetch_half(0)
                fetch_half(1)

                # S.T via PE transposes: st_r[:, jt, :] = S.T[jt*128:(jt+1)*128, :]
                st_r = pa.tile([128, KT, RPC], f32r, name="st_r")
                for jt in range(KT):
                    ps = paps.tile([128, RPC], f32r, name="st_ps")
                    for a in range(ITILES):
                        nc.tensor.transpose(
                            ps[:, a * 128:(a + 1) * 128],
                            s_r[:, a, jt * 128:(jt + 1) * 128],
                            ident,
                        )
                    nc.vector.tensor_copy(st_r[:, jt, :], ps)

                # A.T[kt] = sum_jt W[jt, kt].T-block @ S.T[jt]
                for kt in range(KT):
                    ps = paps.tile([128, RPC], f32, name="at_ps")
                    for jt in range(KT):
                        nc.tensor.matmul(
                            ps,
                            w_r[:, jt, kt * 128:(kt + 1) * 128],
                            st_r[:, jt, :],
                            start=(jt == 0),
                            stop=(jt == KT - 1),
                        )
                    nc.vector.tensor_copy(at_r[:, kt, :], ps)

            # ---------------- Phase B: E = A @ H.T, streamed over seq chunks ----------------
            with tc.tile_pool(name="ebuf", bufs=1) as ebuf, \
                 tc.tile_pool(name="e_ps", bufs=4, space="PSUM") as e_ps:

                E = [ebuf.tile([128, SEQ], f32, name=f"E{i}") for i in range(ITILES)]
                strip = small.tile([128, ITILES, NCHUNK], f32, name="strip")

                # transpose chunk c+1 before chunk c's matmuls: the PSUM->SBUF
                # copies of chunk c then land with a full transpose-block of
                # slack, so the E matmuls never stall on them.
                fetch_half(2)
                fetch_half(3)
                transpose_chunk(0)
                for c in range(NCHUNK):
                    if c + 1 < NCHUNK:
                        transpose_chunk(c + 1)
                        for hh in (2 * c + 4, 2 * c + 5):
                            if hh < NHALF:
                                fetch_half(hh)
                    ht = ht_tiles.pop(c)
                    for i in range(ITILES):
                        eps = e_ps.tile([128, SCHUNK], f32, name="eps")
                        for kb in range(KT):
                            nc.tensor.matmul(
                                eps,
                                at_r[:, kb, i * 128:(i + 1) * 128],
                                ht[:, kb, :],
                                start=(kb == 0),
                                stop=(kb == KT - 1),
                            )
                        nc.scalar.copy(out=E[i][:, c * SCHUNK:(c + 1) * SCHUNK], in_=eps)
                        nc.vector.reduce_max(
                            out=strip[:, i, c:c + 1],
                            in_=E[i][:, c * SCHUNK:(c + 1) * SCHUNK],
                            axis=AXX,
                        )

                # ---------------- Phase C: row softmax over the full seq ----------------
                NQ = 4
                QRT = SEQ // NQ
                negm = small.tile([128, ITILES], f32, name="negm")
                lsum = small.tile([128, ITILES, NQ], f32, name="lsum")
                ltot = small.tile([128, ITILES], f32, name="ltot")
                linv = small.tile([128, ITILES], f32, name="linv")
                for i in range(ITILES):
                    nc.vector.tensor_reduce(
                        out=negm[:, i:i + 1], in_=strip[:, i, :], axis=AXX,
                        op=mybir.AluOpType.max, negate=True,
                    )
                    for q in range(NQ):
                        nc.scalar.activation(
                            out=E[i][:, q * QRT:(q + 1) * QRT],
                            in_=E[i][:, q * QRT:(q + 1) * QRT],
                            func=EXP,
                            bias=negm[:, i:i + 1], scale=1.0,
                            accum_out=lsum[:, i, q:q + 1],
                        )
                    nc.vector.reduce_sum(
                        out=ltot[:, i:i + 1], in_=lsum[:, i, :], axis=AXX,
                    )
                    nc.vector.reciprocal(linv[:, i:i + 1], ltot[:, i:i + 1])
                    for q in range(NQ):
                        sl = slice(q * QRT, (q + 1) * QRT)
                        nc.vector.tensor_scalar_mul(E[i][:, sl], E[i][:, sl], linv[:, i:i + 1])
                        nc.sync.dma_start(out=o_d[i * 128:(i + 1) * 128, sl], in_=E[i][:, sl])

    nc.compile()
    return nc


_NC = None


def _get_nc():
    global _NC
    if _NC is None:
        _NC = _build()
    return _NC


_EYE = np.eye(128, dtype=np.float32)


def _in_maps(out_state, history, W):
    out_state = np.ascontiguousarray(np.asarray(out_state, dtype=np.float32))
    history = np.ascontiguousarray(np.asarray(history, dtype=np.float32))
    W = np.ascontiguousarray(np.asarray(W, dtype=np.float32))
    return [
        {"s": out_state[c * RPC:(c + 1) * RPC], "h": history, "w": W, "eye": _EYE}
        for c in range(NCORES)
    ]


def kernel(out_state, history, W, b):
    nc = _get_nc()
    res = run_bass_kernel_spmd(nc, _in_maps(out_state, history, W), core_ids=list(range(NCORES)))
    return np.concatenate([res.results[c]["o"] for c in range(NCORES)], axis=0)


# revision 22
# speedup vs baseline: 1.0586x; 1.0586x over previous
"""Trainium2 Bass kernel for nn_Attn: softmax(out_state @ (history @ W.T + b).T, axis=1).

Key algebra: E = out_state @ proj.T = (out_state @ W) @ history.T + (out_state @ b) 1^T.
The bias contributes a per-row constant, which softmax is invariant to, so it is
dropped entirely.  Per core (1/8 of out_state rows):
    A.T = W.T @ S.T        (PE, fp32r)
    E   = A @ H.T          (PE, fp32r; H transposed on-chip via PE transpose-mode)
    out = softmax(E, 1)    (DVE max/scale, ACT exp with per-row bias + sum accumulation)

fp32r operands are produced by SWDGE cast-DMAs (W, S, H, identity arrive
pre-rounded) or by the PSUM->SBUF copies that must happen anyway.
"""

import numpy as np

import concourse.bacc as bacc
import concourse.bass as bass
import concourse.tile as tile
from concourse import mybir
from concourse.bass_utils import run_bass_kernel_spmd

STATE, SEQ, HID = 4096, 8192, 1024
NCORES = 8
RPC = STATE // NCORES          # 512 out_state rows per core
ITILES = RPC // 128            # 4
KT = HID // 128                # 8 contraction tiles
SCHUNK = 512                   # seq columns per streamed chunk
NCHUNK = SEQ // SCHUNK         # 16
SSUB = SCHUNK // 128           # 4 row sub-tiles per chunk
NHALF = SEQ // 256             # H streamed in 256-row half-chunks

f32 = mybir.dt.float32
f32r = mybir.dt.float32r
AXX = mybir.AxisListType.X
EXP = mybir.ActivationFunctionType.Exp


def _build():
    nc = bacc.Bacc("TRN2", target_bir_lowering=False, debug=False)
    s_d = nc.dram_tensor("s", [RPC, HID], f32, kind="ExternalInput").ap()
    h_d = nc.dram_tensor("h", [SEQ, HID], f32, kind="ExternalInput").ap()
    w_d = nc.dram_tensor("w", [HID, HID], f32, kind="ExternalInput").ap()
    eye_d = nc.dram_tensor("eye", [128, 128], f32, kind="ExternalInput").ap()
    o_d = nc.dram_tensor("o", [RPC, SEQ], f32, kind="ExternalOutput").ap()

    with tile.TileContext(nc) as tc:
        with tc.tile_pool(name="persist", bufs=1) as persist, \
             tc.tile_pool(name="hraw", bufs=2) as hraw_p, \
             tc.tile_pool(name="htp", bufs=2) as ht_p, \
             tc.tile_pool(name="small", bufs=1) as small:

            # fp32r identity for transpose-mode matmuls
            ident = persist.tile([128, 128], f32r, name="ident")
            nc.gpsimd.dma_start(out=ident, in_=eye_d)
            # A.T laid out [k_partition, kt, i], fp32r (stationary operand of E matmuls)
            at_r = persist.tile([128, KT, RPC], f32r, name="at_r")

            half_tiles = {}

            def fetch_half(hh):
                hr = hraw_p.tile([128, 2, HID], f32r, name="hr")
                nc.gpsimd.dma_start(
                    out=hr,
                    in_=h_d[hh * 256:(hh + 1) * 256, :].rearrange(
                        "(a p) k -> p a k", p=128
                    ),
                )
                half_tiles[hh] = hr

            ht_tiles = {}
            hr_pair = {}
            tp_ps_cell = [None]

            def transpose_chunk_half(c, phase):
                """Transpose kb range [4*phase, 4*phase+4) of chunk c."""
                if phase == 0:
                    hr_pair[c] = (half_tiles.pop(2 * c), half_tiles.pop(2 * c + 1))
                    ht_tiles[c] = ht_p.tile([128, KT, SCHUNK], f32r, name="ht")
                hr0, hr1 = hr_pair[c]
                ht = ht_tiles[c]
                tp_ps = tp_ps_cell[0]
                for kb in range(4 * phase, 4 * phase + 4):
                    tp = tp_ps.tile([128, SCHUNK], f32r, name="tp")
                    for a in range(SSUB):
                        src = hr0 if a < 2 else hr1
                        nc.tensor.transpose(
                            tp[:, a * 128:(a + 1) * 128],
                            src[:, a % 2, kb * 128:(kb + 1) * 128],
                            ident,
                        )
                    # alternate PSUM->SBUF copies between DVE and ACT
                    if kb % 2 == 1:
                        nc.scalar.copy(out=ht[:, kb, :], in_=tp)
                    else:
                        nc.vector.tensor_copy(ht[:, kb, :], tp)
                if phase == 1:
                    hr_pair.pop(c)

            def transpose_chunk(c):
                transpose_chunk_half(c, 0)
                transpose_chunk_half(c, 1)

            # ---------------- Phase A: A.T = W.T @ S.T ----------------
            with tc.tile_pool(name="pa", bufs=1) as pa, \
                 tc.tile_pool(name="pa_ps", bufs=2, space="PSUM") as paps:
                s_r = pa.tile([128, ITILES, HID], f32r, name="s_r")
                nc.gpsimd.dma_start(out=s_r, in_=s_d.rearrange("(a p) k -> p a k", p=128))
                w_r = pa.tile([128, KT, HID], f32r, name="w_r")
                # split so A.T for the low kt half can start sooner
                nc.gpsimd.dma_start(
                    out=w_r[:, :, 0:HID // 2],
                    in_=w_d[:, 0:HID // 2].rearrange("(t p) k -> p t k", p=128),
                )
                nc.gpsimd.dma_start(
                    out=w_r[:, :, HID // 2:],
                    in_=w_d[:, HID // 2:].rearrange("(t p) k -> p t k", p=128),
                )
                fetch_half(0)
                fetch_half(1)

                # S.T via PE transposes: st_r[:, jt, :] = S.T[jt*128:(jt+1)*128, :]
                st_r = pa.tile([128, KT, RPC], f32r, name="st_r")
                for jt in range(KT):
                    ps = paps.tile([128, RPC], f32r, name="st_ps")
                    for a in range(ITILES):
                        nc.tensor.transpose(
                            ps[:, a * 128:(a + 1) * 128],
                            s_r[:, a, jt * 128:(jt + 1) * 128],
                            ident,
                        )
                    nc.vector.tensor_copy(st_r[:, jt, :], ps)

                # A.T[kt] = sum_jt W[jt, kt].T-block @ S.T[jt]
                for kt in range(KT):
                    ps = paps.tile([128, RPC], f32, name="at_ps")
                    for jt in range(KT):
                        nc.tensor.matmul(
                            ps,
                            w_r[:, jt, kt * 128:(kt + 1) * 128],
                            st_r[:, jt, :],
                            start=(jt == 0),
                            stop=(jt == KT - 1),
                        )
                    nc.vector.tensor_copy(at_r[:, kt, :], ps)

            # ---------------- Phase B: E = A @ H.T, streamed over seq chunks ----------------
            with tc.tile_pool(name="ebuf", bufs=1) as ebuf, \
                 tc.tile_pool(name="tp_ps", bufs=6, space="PSUM") as tp_ps, \
                 tc.tile_pool(name="e_ps", bufs=2, space="PSUM") as e_ps:
                tp_ps_cell[0] = tp_ps

                E = [ebuf.tile([128, SEQ], f32, name=f"E{i}") for i in range(ITILES)]
                strip = small.tile([128, ITILES, NCHUNK], f32, name="strip")

                # transpose chunk c+1 interleaved with chunk c's matmuls: each
                # half-block of transposes is followed by two E matmul groups,
                # so the PSUM->SBUF copies always have matmul-time slack and
                # neither PE nor the copy engines ever stall.
                fetch_half(2)
                fetch_half(3)
                transpose_chunk(0)

                def e_group(c, i, ht):
                    # E chunk accumulates in PSUM; the PSUM->SBUF copy IS the
                    # exp (ACT is rate-1 for any function): store
                    # P~ = exp(E - M_c) with M_c the chunk's own row max
                    # (strip holds -M_c), plus the chunk sum s_c.  The global
                    # correction exp(M_c - m)/l is applied in phase C.
                    eps = e_ps.tile([128, SCHUNK], f32, name="eps")
                    for kb in range(KT):
                        nc.tensor.matmul(
                            eps,
                            at_r[:, kb, i * 128:(i + 1) * 128],
                            ht[:, kb, :],
                            start=(kb == 0),
                            stop=(kb == KT - 1),
                        )
                    nc.vector.tensor_reduce(
                        out=strip[:, i, c:c + 1], in_=eps, axis=AXX,
                        op=mybir.AluOpType.max, negate=True,
                    )
                    nc.scalar.activation(
                        out=E[i][:, c * SCHUNK:(c + 1) * SCHUNK], in_=eps,
                        func=EXP, bias=strip[:, i, c:c + 1], scale=1.0,
                        accum_out=ssum[:, i, c:c + 1],
                    )

                for c in range(NCHUNK):
                    ht = ht_tiles.pop(c)
                    for phase in range(2):
                        if c + 1 < NCHUNK:
                            transpose_chunk_half(c + 1, phase)
                        for i in (0, 1) if phase == 0 else (2, 3):
                            e_group(c, i, ht)
                    for hh in (2 * c + 4, 2 * c + 5):
                        if hh < NHALF:
                            fetch_half(hh)

                # ---------------- Phase C: row softmax over the full seq ----------------
                NQ = 4
                QRT = SEQ // NQ
                negm = small.tile([128, ITILES], f32, name="negm")
                lsum = small.tile([128, ITILES, NQ], f32, name="lsum")
                ltot = small.tile([128, ITILES], f32, name="ltot")
                linv = small.tile([128, ITILES], f32, name="linv")
                for i in range(ITILES):
                    nc.vector.tensor_reduce(
                        out=negm[:, i:i + 1], in_=strip[:, i, :], axis=AXX,
                        op=mybir.AluOpType.max, negate=True,
                    )
                    for q in range(NQ):
                        nc.scalar.activation(
                            out=E[i][:, q * QRT:(q + 1) * QRT],
                            in_=E[i][:, q * QRT:(q + 1) * QRT],
                            func=EXP,
                            bias=negm[:, i:i + 1], scale=1.0,
                            accum_out=lsum[:, i, q:q + 1],
                        )
                    nc.vector.reduce_sum(
                        out=ltot[:, i:i + 1], in_=lsum[:, i, :], axis=AXX,
                    )
                    nc.vector.reciprocal(linv[:, i:i + 1], ltot[:, i:i + 1])
                    for q in range(NQ):
                        sl = slice(q * QRT, (q + 1) * QRT)
                        nc.vector.tensor_scalar_mul(E[i][:, sl], E[i][:, sl], linv[:, i:i + 1])
                        nc.sync.dma_start(out=o_d[i * 128:(i + 1) * 128, sl], in_=E[i][:, sl])

    nc.compile()
    return nc


_NC = None


def _get_nc():
    global _NC
    if _NC is None:
        _NC = _build()
    return _NC


_EYE = np.eye(128, dtype=np.float32)


def _in_maps(out_state, history, W):
    out_state = np.ascontiguousarray(np.asarray(out_state, dtype=np.float32))
    history = np.ascontiguousarray(np.asarray(history, dtype=np.float32))
    W = np.ascontiguousarray(np.asarray(W, dtype=np.float32))
    return [
        {"s": out_state[c * RPC:(c + 1) * RPC], "h": history, "w": W, "eye": _EYE}
        for c in range(NCORES)
    ]


def kernel(out_state, history, W, b):
    nc = _get_nc()
    res = run_bass_kernel_spmd(nc, _in_maps(out_state, history, W), core_ids=list(range(NCORES)))
    return np.concatenate([res.results[c]["o"] for c in range(NCORES)], axis=0)


# revision 24
# speedup vs baseline: 1.0789x; 1.0192x over previous
"""Trainium2 Bass kernel for nn_Attn: softmax(out_state @ (history @ W.T + b).T, axis=1).

Key algebra: E = out_state @ proj.T = (out_state @ W) @ history.T + (out_state @ b) 1^T.
The bias contributes a per-row constant, which softmax is invariant to, so it is
dropped entirely.  Per core (1/8 of out_state rows):
    A.T = W.T @ S.T        (PE, fp32r)
    E   = A @ H.T          (PE, fp32r; H transposed on-chip via PE transpose-mode)
    out = softmax(E, 1)    (DVE max/scale, ACT exp with per-row bias + sum accumulation)

fp32r operands are produced by SWDGE cast-DMAs (W, S, H, identity arrive
pre-rounded) or by the PSUM->SBUF copies that must happen anyway.
"""

import numpy as np

import concourse.bacc as bacc
import concourse.bass as bass
import concourse.tile as tile
from concourse import mybir
from concourse.bass_utils import run_bass_kernel_spmd

STATE, SEQ, HID = 4096, 8192, 1024
NCORES = 8
RPC = STATE // NCORES          # 512 out_state rows per core
ITILES = RPC // 128            # 4
KT = HID // 128                # 8 contraction tiles
SCHUNK = 512                   # seq columns per streamed chunk
NCHUNK = SEQ // SCHUNK         # 16
SSUB = SCHUNK // 128           # 4 row sub-tiles per chunk
NHALF = SEQ // 256             # H streamed in 256-row half-chunks

f32 = mybir.dt.float32
f32r = mybir.dt.float32r
AXX = mybir.AxisListType.X
EXP = mybir.ActivationFunctionType.Exp


def _build():
    nc = bacc.Bacc("TRN2", target_bir_lowering=False, debug=False)
    s_d = nc.dram_tensor("s", [RPC, HID], f32, kind="ExternalInput").ap()
    h_d = nc.dram_tensor("h", [SEQ, HID], f32, kind="ExternalInput").ap()
    w_d = nc.dram_tensor("w", [HID, HID], f32, kind="ExternalInput").ap()
    eye_d = nc.dram_tensor("eye", [128, 128], f32, kind="ExternalInput").ap()
    o_d = nc.dram_tensor("o", [RPC, SEQ], f32, kind="ExternalOutput").ap()

    with tile.TileContext(nc) as tc:
        with tc.tile_pool(name="persist", bufs=1) as persist, \
             tc.tile_pool(name="hraw", bufs=2) as hraw_p, \
             tc.tile_pool(name="htp", bufs=2) as ht_p, \
             tc.tile_pool(name="small", bufs=1) as small:

            # fp32r identity for transpose-mode matmuls
            ident = persist.tile([128, 128], f32r, name="ident")
            nc.gpsimd.dma_start(out=ident, in_=eye_d)
            # A.T laid out [k_partition, kt, i], fp32r (stationary operand of E matmuls)
            at_r = persist.tile([128, KT, RPC], f32r, name="at_r")

            half_tiles = {}

            def fetch_half(hh):
                hr = hraw_p.tile([128, 2, HID], f32r, name="hr")
                nc.gpsimd.dma_start(
                    out=hr,
                    in_=h_d[hh * 256:(hh + 1) * 256, :].rearrange(
                        "(a p) k -> p a k", p=128
                    ),
                )
                half_tiles[hh] = hr

            ht_tiles = {}
            hr_pair = {}
            tp_ps_cell = [None]

            def transpose_chunk_half(c, phase):
                """Transpose kb range [4*phase, 4*phase+4) of chunk c."""
                if phase == 0:
                    hr_pair[c] = (half_tiles.pop(2 * c), half_tiles.pop(2 * c + 1))
                    ht_tiles[c] = ht_p.tile([128, KT, SCHUNK], f32r, name="ht")
                hr0, hr1 = hr_pair[c]
                ht = ht_tiles[c]
                tp_ps = tp_ps_cell[0]
                for kb in range(4 * phase, 4 * phase + 4):
                    tp = tp_ps.tile([128, SCHUNK], f32r, name="tp")
                    for a in range(SSUB):
                        src = hr0 if a < 2 else hr1
                        nc.tensor.transpose(
                            tp[:, a * 128:(a + 1) * 128],
                            src[:, a % 2, kb * 128:(kb + 1) * 128],
                            ident,
                        )
                    # alternate PSUM->SBUF copies between DVE and ACT
                    if kb % 2 == 1:
                        nc.scalar.copy(out=ht[:, kb, :], in_=tp)
                    else:
                        nc.vector.tensor_copy(ht[:, kb, :], tp)
                if phase == 1:
                    hr_pair.pop(c)

            def transpose_chunk(c):
                transpose_chunk_half(c, 0)
                transpose_chunk_half(c, 1)

            # ---------------- Phase A: A.T = W.T @ S.T ----------------
            with tc.tile_pool(name="pa", bufs=1) as pa, \
                 tc.tile_pool(name="pa_ps", bufs=2, space="PSUM") as paps:
                s_r = pa.tile([128, ITILES, HID], f32r, name="s_r")
                nc.gpsimd.dma_start(out=s_r, in_=s_d.rearrange("(a p) k -> p a k", p=128))
                w_r = pa.tile([128, KT, HID], f32r, name="w_r")
                # split so A.T for the low kt half can start sooner
                nc.gpsimd.dma_start(
                    out=w_r[:, :, 0:HID // 2],
                    in_=w_d[:, 0:HID // 2].rearrange("(t p) k -> p t k", p=128),
                )
                nc.gpsimd.dma_start(
                    out=w_r[:, :, HID // 2:],
                    in_=w_d[:, HID // 2:].rearrange("(t p) k -> p t k", p=128),
                )
                fetch_half(0)
                fetch_half(1)

                # S.T via PE transposes: st_r[:, jt, :] = S.T[jt*128:(jt+1)*128, :]
                st_r = pa.tile([128, KT, RPC], f32r, name="st_r")
                for jt in range(KT):
                    ps = paps.tile([128, RPC], f32r, name="st_ps")
                    for a in range(ITILES):
                        nc.tensor.transpose(
                            ps[:, a * 128:(a + 1) * 128],
                            s_r[:, a, jt * 128:(jt + 1) * 128],
                            ident,
                        )
                    nc.vector.tensor_copy(st_r[:, jt, :], ps)

                # A.T[kt] = sum_jt W[jt, kt].T-block @ S.T[jt]
                for kt in range(KT):
                    ps = paps.tile([128, RPC], f32, name="at_ps")
                    for jt in range(KT):
                        nc.tensor.matmul(
                            ps,
                            w_r[:, jt, kt * 128:(kt + 1) * 128],
                            st_r[:, jt, :],
                            start=(jt == 0),
                            stop=(jt == KT - 1),
                        )
                    nc.vector.tensor_copy(at_r[:, kt, :], ps)

            # ---------------- Phase B: E = A @ H.T, streamed over seq chunks ----------------
            with tc.tile_pool(name="ebuf", bufs=1) as ebuf, \
                 tc.tile_pool(name="tp_ps", bufs=6, space="PSUM") as tp_ps, \
                 tc.tile_pool(name="e_ps", bufs=2, space="PSUM") as e_ps:
                tp_ps_cell[0] = tp_ps

                E = [ebuf.tile([128, SEQ], f32, name=f"E{i}") for i in range(ITILES)]
                strip = small.tile([128, ITILES, NCHUNK], f32, name="strip")
                ssum = small.tile([128, ITILES, NCHUNK], f32, name="ssum")

                # transpose chunk c+1 interleaved with chunk c's matmuls: each
                # half-block of transposes is followed by two E matmul groups,
                # so the PSUM->SBUF copies always have matmul-time slack and
                # neither PE nor the copy engines ever stall.
                fetch_half(2)
                fetch_half(3)
                transpose_chunk(0)

                def e_group(c, i, ht):
                    # E chunk accumulates in PSUM; the PSUM->SBUF copy IS the
                    # exp (ACT is rate-1 for any function): store
                    # P~ = exp(E - M_c) with M_c the chunk's own row max
                    # (strip holds -M_c), plus the chunk sum s_c.  The global
                    # correction exp(M_c - m)/l is applied in phase C.
                    eps = e_ps.tile([128, SCHUNK], f32, name="eps")
                    for kb in range(KT):
                        nc.tensor.matmul(
                            eps,
                            at_r[:, kb, i * 128:(i + 1) * 128],
                            ht[:, kb, :],
                            start=(kb == 0),
                            stop=(kb == KT - 1),
                        )
                    nc.vector.tensor_reduce(
                        out=strip[:, i, c:c + 1], in_=eps, axis=AXX,
                        op=mybir.AluOpType.max, negate=True,
                    )
                    nc.scalar.activation(
                        out=E[i][:, c * SCHUNK:(c + 1) * SCHUNK], in_=eps,
                        func=EXP, bias=strip[:, i, c:c + 1], scale=1.0,
                        accum_out=ssum[:, i, c:c + 1],
                    )

                for c in range(NCHUNK):
                    ht = ht_tiles.pop(c)
                    for phase in range(2):
                        if c + 1 < NCHUNK:
                            transpose_chunk_half(c + 1, phase)
                        for i in (0, 1) if phase == 0 else (2, 3):
                            e_group(c, i, ht)
                    for hh in (2 * c + 4, 2 * c + 5):
                        if hh < NHALF:
                            fetch_half(hh)

                # ---------------- Phase C: global softmax correction ----------------
                # E holds exp(E - M_c) per chunk; strip holds -M_c, ssum holds
                # s_c = sum exp(E - M_c).  m = max_c M_c, f_c = exp(M_c - m),
                # l = sum_c s_c f_c, and the final scale is g_c = f_c / l.
                negm = small.tile([128, ITILES], f32, name="negm")
                fbuf = small.tile([128, ITILES, NCHUNK], f32, name="fbuf")
                prod = small.tile([128, ITILES, NCHUNK], f32, name="prod")
                gbuf = small.tile([128, ITILES, NCHUNK], f32, name="gbuf")
                ltot = small.tile([128, ITILES], f32, name="ltot")
                linv = small.tile([128, ITILES], f32, name="linv")
                CPQ = 4  # chunks per output DMA
                for i in range(ITILES):
                    nc.vector.tensor_reduce(
                        out=negm[:, i:i + 1], in_=strip[:, i, :], axis=AXX,
                        op=mybir.AluOpType.min,
                    )
                    # f_c = exp(-strip_c + negm) = exp(M_c - m)
                    nc.scalar.activation(
                        out=fbuf[:, i, :], in_=strip[:, i, :], func=EXP,
                        bias=negm[:, i:i + 1], scale=-1.0,
                    )
                    nc.vector.tensor_tensor(
                        out=prod[:, i, :], in0=ssum[:, i, :], in1=fbuf[:, i, :],
                        op=mybir.AluOpType.mult,
                    )
                    nc.vector.reduce_sum(
                        out=ltot[:, i:i + 1], in_=prod[:, i, :], axis=AXX,
                    )
                    nc.vector.reciprocal(linv[:, i:i + 1], ltot[:, i:i + 1])
                    nc.vector.tensor_scalar_mul(
                        gbuf[:, i, :], fbuf[:, i, :], linv[:, i:i + 1],
                    )
                    for q in range(NCHUNK // CPQ):
                        for cc in range(CPQ * q, CPQ * q + CPQ):
                            sl = slice(cc * SCHUNK, (cc + 1) * SCHUNK)
                            if cc % 2 == 0:
                                nc.vector.tensor_scalar_mul(
                                    E[i][:, sl], E[i][:, sl], gbuf[:, i, cc:cc + 1],
                                )
                            else:
                                nc.scalar.activation(
                                    out=E[i][:, sl], in_=E[i][:, sl],
                                    func=mybir.ActivationFunctionType.Copy,
                                    bias=0.0, scale=gbuf[:, i, cc:cc + 1],
                                )
                        qsl = slice(q * CPQ * SCHUNK, (q + 1) * CPQ * SCHUNK)
                        nc.sync.dma_start(out=o_d[i * 128:(i + 1) * 128, qsl], in_=E[i][:, qsl])

    nc.compile()
    return nc


_NC = None


def _get_nc():
    global _NC
    if _NC is None:
        _NC = _build()
    return _NC


_EYE = np.eye(128, dtype=np.float32)


def _in_maps(out_state, history, W):
    out_state = np.ascontiguousarray(np.asarray(out_state, dtype=np.float32))
    history = np.ascontiguousarray(np.asarray(history, dtype=np.float32))
    W = np.ascontiguousarray(np.asarray(W, dtype=np.float32))
    return [
        {"s": out_state[c * RPC:(c + 1) * RPC], "h": history, "w": W, "eye": _EYE}
        for c in range(NCORES)
    ]


def kernel(out_state, history, W, b):
    nc = _get_nc()
    res = run_bass_kernel_spmd(nc, _in_maps(out_state, history, W), core_ids=list(range(NCORES)))
    return np.concatenate([res.results[c]["o"] for c in range(NCORES)], axis=0)
